# revision 33
# baseline (speedup 1.0000x reference)
"""Trainium2 Bass kernel for nn_Attn_48137993453608.

Module: Y = X@W1.T+b1 -> split Q,K,V -> w = softmax((Q_h^T K_h)/sqrt(S))
        (attention over the DH=64 dim, contracting S) -> out = w @ V_h^T
        -> raw memory-order reshape [B,H,DH,S]->[B,S,D] -> @ W2.T + b2.

Sharding: 8 cores = 4 batch x 2 head-groups (8 heads each). Each core owns a
contiguous [1024, 1024] block of the output (rows i = 128*h + 2*d + (s>=1024)
for its heads), so no collectives are needed.

Key reassociation: the final projection contracts the attention output over
j = s mod 1024, and the attention output is linear in V, so

  F_un[c2, n] = sum_j OT_un[j, c2] W2T[j, n]
              = sum_e expw[e, c2] * G[e, n],
  G_{p,half}[e, n] = sum_j V[half*1024+j, p*128+e] * W2T[j, n].

G is softmax-independent, so nearly all of the output-projection FLOPs run
inside phase 1's dense matmul stream; only a single [128x128]x[128x512]
matmul per (pair, half, nh) remains after the softmax.  b2 is folded into
G (G' = G + b2): since the normalized softmax weights sum to 1, the +b2
surfaces exactly once in the output after the rZ scaling.

Per-core dataflow:
  1. Y[s, :1536] = Xb @ Wqkv.T + b     (Q | K | V columns, local heads)
  2. wT accumulation packed per head-PAIR: one [128s,128]x[128s,128] matmul
     per (s-tile, pair) -> off-diagonal 64x64 blocks are junk, ignored.
     The pair matmuls are LDWEIGHTS-bound when bunched, so each chunk's
     are queued and drained one-per-group into the NEXT chunk's dense
     N=512 stream (chunk 7's drain before exp emission -- the exp RAW
     dependency binds at emission time).
  3. phase 2: exp (ACT) + all Z ones-matmuls + one reciprocal up front
     (so F has no late deps); then per pair p: [8x G-matmul accum + DVE
     eviction (+b2, ->bf16)]; F(p-1) is emitted at high scheduler
     priority between the two halves of G(p) so stores spread through
     the G pass; only F(3) tails.  F evictions scale by rZ
     (per-partition) split across ACT and DVE; one [128,1024] bf16 store
     per (p,half) -- fewer dma_starts/semaphores shorten the serialized
     epilogue.  The last G block skips the b2-fold (plain ACT Copy,
     parallel with DVE) and its F eviction restores b2 via DVE
     scalar_tensor_tensor, shortening the final dependency chain.
     Output bf16; host upcasts.

Startup engineering (the startup is chip-HBM-bound: 8 cores x ~3.9MB of
wqkv/xbt/bias must land before the QKV stream runs free):
  - wqkv split into 2-kb-block DMAs across all three DMA-capable engines
    (SP/Activation/gpsimd); each dma_start costs its engine ~0.7us of
    sequencer time, so queue assignment and order matter.
  - X[b].T is host-reordered chunk-major so chunk loads are 4KB-line
    descriptors (512B lines lose DMA-engine arbitration 7:1).
  - w2t/b2 (2.25MB, needed only at ~115us) are gated behind chunk-2
    progress via WAR data dependencies (dummy DVE reads of both DMA dest
    regions that also read a chunk-2 tile) -- engine-order blockers alone
    get hoisted by the Tile scheduler.
  - ~6us of junk warmup matmuls on a constant tile keep the PE busy while
    the first weights stream in (HAM clock-gate warm-up).

Precision: the whole matmul stream is bf16 with fp32 PSUM accumulation and
bf16 bias adds (biases are ~0.01, rounding is negligible).  Logits are soft
(|logit| <= ~6) and Z is computed from the same bf16 exp values used in the
F matmul.  bf16 output rounding adds ~0.2%.  Measured ~4.9e-3 rel-l2 error
vs the fp32 oracle.  Measured ~151-152us HW exec at the 2.4GHz PE state (the
shared part sporadically runs whole executions in a ~2.0GHz P-state, which
reads ~15% slower for any kernel version).
"""

import os
import sys

for _p in ("/opt/trn_rl_repo",):
    if _p not in sys.path and os.path.isdir(_p):
        sys.path.insert(0, _p)

import ml_dtypes
import numpy as np

import concourse.bass as bass
import concourse.bacc as bacc
import concourse.mybir as mybir
import concourse.tile as tile
from concourse.bass_utils import run_bass_kernel_spmd

B, S, D, H = 4, 2048, 1024, 16
DH = D // H          # 64
NH = 8               # heads per core
SCALE = 1.0 / float(np.sqrt(np.float32(S)))

F32 = mybir.dt.float32
BF16 = mybir.dt.bfloat16

S_CHUNK = 256                 # s columns of X^T staged per iteration
N_SCHUNKS = S // S_CHUNK      # 8
ST_PER_CHUNK = S_CHUNK // 128 # 2


def build_nc():
    nc = bacc.Bacc("TRN2", target_bir_lowering=False, debug=False)

    # X[b].T, host-reordered chunk-major: row c*128+p, col kb*256+s' holds
    # X[b].T[kb*128+p, c*256+s'].  A chunk load is then a plain [128, 2048]
    # row-slice with 4KB-contiguous per-partition lines (the natural [d, s]
    # layout would give 512B descriptors, which lose DMA-engine round-robin
    # arbitration against the 3KB wqkv descriptors during the startup crunch).
    xbt = nc.dram_tensor("xbt", [D, S], BF16, kind="ExternalInput")
    # wq/wk/wv host-repacked [128, kb*512+c]: 8KB-contiguous partition rows so
    # the startup loads use fat (4KB-line) descriptors on a single ring.
    wq_d = nc.dram_tensor("wq", [128, 4096], BF16, kind="ExternalInput")
    wk_d = nc.dram_tensor("wk", [128, 4096], BF16, kind="ExternalInput")
    wv_d = nc.dram_tensor("wv", [128, 4096], BF16, kind="ExternalInput")
    # b2 host-expanded to 128 partitions (plain lines; broadcast-DMA
    # descriptors are pathologically slow).  bqkv stays a 3KB row and is
    # broadcast on-chip by a ones-matmul -- a 384KB DMA would compete with
    # the startup-critical weight stream.
    bqkv = nc.dram_tensor("bqkv", [1, 1536], BF16, kind="ExternalInput")
    w2t = nc.dram_tensor("w2t", [D, 1024], BF16, kind="ExternalInput")     # W2.T
    b2 = nc.dram_tensor("b2", [128, 1024], BF16, kind="ExternalInput")
    out = nc.dram_tensor("out", [1024, 1024], BF16, kind="ExternalOutput")

    # per-chunk col order [st, kb, 128] so chunk 0 can split into two
    # 2KB-line st-DMAs (the first QKV group needs only st=0).
    xbt_v = xbt[:].rearrange("(c p) (st kb s) -> c p st kb s",
                             p=128, st=ST_PER_CHUNK, s=128)
    wq_v = wq_d[:].rearrange("p (kb c) -> p kb c", c=512)      # [128, 8, 512]
    wk_v = wk_d[:].rearrange("p (kb c) -> p kb c", c=512)
    wv_v = wv_d[:].rearrange("p (kb c) -> p kb c", c=512)
    w2t_v = w2t[:].rearrange("(kb p) c -> p kb c", p=128)      # [128, 8, 1024]
    # output rows r = 256*p + 128*g + 2*d + half
    out_v = out[:].rearrange("(p g d h) n -> p g d h n", p=4, g=2, d=64, h=2)

    with tile.TileContext(nc) as tc:
        with (
            tc.tile_pool(name="const", bufs=1) as const,
            tc.tile_pool(name="xin", bufs=3) as xin,
            tc.tile_pool(name="ywork", bufs=8) as ywork,
            tc.tile_pool(name="vstore", bufs=1) as vstore,
            tc.tile_pool(name="attn", bufs=1) as attn,
            tc.tile_pool(name="fout", bufs=4) as fout,
            tc.tile_pool(name="psacc", bufs=3, space="PSUM") as psacc,
            tc.tile_pool(name="pswt", bufs=1, space="PSUM") as pswt,
            tc.tile_pool(name="psg", bufs=2, space="PSUM") as psg,
            tc.tile_pool(name="pszr", bufs=2, space="PSUM") as pszr,
        ):
            # ---------------- phase-1 loads --------------------------------
            # Startup is per-core DMA-bandwidth bound (~358 GB/s): the whole
            # critical byte stream goes on ONE ring (sync) in exact
            # consumption order, so transfers complete FIFO with no
            # round-robin competition:
            #   sync:   xbt c0, wq a/b, wk a/b, wv a/b, xbt c1..c7, stores
            #   scalar: w2t half (gated at sc==2), stores
            #   gpsimd: b_bc, b2_bc + w2t half (gated), stores
            # Tile dependency tracking is per-TILE, not per-slice: a matmul
            # reading any slice waits for EVERY dma writing that tile.  So
            # each startup DMA gets its own tile, sized to what the first
            # consumer group needs.
            c0st_sb = [const.tile([128, 8, 128], BF16, name=f"c0st{st}")
                       for st in range(ST_PER_CHUNK)]
            wq_tiles = [const.tile([128, 2, 512], BF16, name=f"wq{i}")
                        for i in range(4)]
            wk_tiles = [const.tile([128, 4, 512], BF16, name=f"wk{i}")
                        for i in range(2)]
            wv_tiles = [const.tile([128, 4, 512], BF16, name=f"wv{i}")
                        for i in range(2)]
            b_bc = const.tile([128, 1536], BF16)
            b_row = const.tile([1, 1536], BF16)
            ones_row = const.tile([1, 128], BF16)
            nc.gpsimd.dma_start(out=b_row[:], in_=bqkv[:])
            nc.vector.memset(ones_row[:], 1.0)
            # fine-grained FIFO on the sync ring, matched to the sc=0 group
            # order (Q,Q,K,K,V,V): compute starts once c0-st0 + wq kb0-1
            # (~512KB) have landed instead of waiting for full 1MB slabs.
            nc.sync.dma_start(out=c0st_sb[0][:], in_=xbt_v[0, :, 0])
            nc.sync.dma_start(out=wq_tiles[0][:], in_=wq_v[:, 0:2, :])
            nc.sync.dma_start(out=wq_tiles[1][:], in_=wq_v[:, 2:4, :])
            nc.sync.dma_start(out=c0st_sb[1][:], in_=xbt_v[0, :, 1])
            nc.sync.dma_start(out=wq_tiles[2][:], in_=wq_v[:, 4:6, :])
            nc.sync.dma_start(out=wq_tiles[3][:], in_=wq_v[:, 6:8, :])
            nc.sync.dma_start(out=wk_tiles[0][:], in_=wk_v[:, 0:4, :])
            nc.sync.dma_start(out=wk_tiles[1][:], in_=wk_v[:, 4:8, :])
            nc.sync.dma_start(out=wv_tiles[0][:], in_=wv_v[:, 0:4, :])
            nc.sync.dma_start(out=wv_tiles[1][:], in_=wv_v[:, 4:8, :])

            def w_ap(nh, kb):
                if nh == 0:
                    return wq_tiles[kb >> 1][:, kb & 1, :]
                t = wk_tiles if nh == 1 else wv_tiles
                return t[kb >> 2][:, kb & 3, :]

            ones_sb = const.tile([128, 1], BF16)
            nc.vector.memset(ones_sb[:], 1.0)

            # HAM warmup: junk matmuls on a constant tile so the PE
            # clock-gate is at 8/8 before the first DMA-paced real matmuls.
            # Sized to end right as xbt c0 + wq land (~10.5us): too long and
            # the real stream queues behind junk, too short and the clock
            # re-gates during the wait.
            warm_sb = const.tile([128, 512], BF16)
            nc.vector.memset(warm_sb[:], 0.5)
            warm_ps = pszr.tile([128, 512], F32, tag="z")
            N_WARM = 12
            for i in range(N_WARM):
                nc.tensor.matmul(
                    warm_ps[:, 0:256],
                    lhsT=warm_sb[:, 0:128],
                    rhs=warm_sb[:, 0:256],
                    start=(i == 0),
                    stop=(i == N_WARM - 1),
                )

            # on-chip bias broadcast: b_bc[p, c] = b_row[0, c] via a
            # contract-1 ones-matmul (3KB DMA instead of 384KB); runs right
            # after warmup while the PE waits for weights anyway.
            for i in range(3):
                ps_b = pszr.tile([128, 512], F32, tag="z", name=f"psb{i}")
                nc.tensor.matmul(
                    ps_b[:],
                    lhsT=ones_row[:],
                    rhs=b_row[:, i * 512:(i + 1) * 512],
                )
                nc.scalar.activation(
                    out=b_bc[:, i * 512:(i + 1) * 512],
                    in_=ps_b[:],
                    func=mybir.ActivationFunctionType.Copy,
                )

            # V (natural [s, vrow] layout) persists until the G pass;
            # wT accumulates across all s.
            v_sb = vstore.tile([128, 16, 512], BF16)
            psum_wt = pswt.tile([128, 512], F32)        # [e2(g*64+e), pair*128+c2]

            # w2t / b2 are only needed at the G pass (~115us); keep their
            # 2.25MB off the HBM-saturated startup window by gating the
            # issuing engines on a chunk-2 tile (tiny Copy blockers).
            w2_sb = const.tile([128, 8, 1024], BF16)
            b2_bc = const.tile([128, 1024], BF16)
            blk_sb = const.tile([1, 2], BF16)

            # wT pair-matmuls are LDWEIGHTS-bound when bunched (~165ns each);
            # spread them one-at-a-time into the NEXT chunk's dense N=512
            # stream (weight loads hide under the 213ns matmuls).  Chunk 7's
            # spill into the first G groups.  ywork holds 4 chunks of yqk
            # tiles so the one-chunk lag is safe.
            wt_first = [True]
            wt_queue = []

            def push_wt(yqk_sb, p, is_last):
                def go():
                    nc.tensor.matmul(
                        psum_wt[:, p * 128:(p + 1) * 128],
                        lhsT=yqk_sb[:, 512 + p * 128:512 + (p + 1) * 128],
                        rhs=yqk_sb[:, p * 128:(p + 1) * 128],
                        start=wt_first[0],
                        stop=is_last,
                        skip_group_check=True,
                    )
                    wt_first[0] = False
                wt_queue.append(go)

            def drain_wt(n):
                for _ in range(min(n, len(wt_queue))):
                    wt_queue.pop(0)()

            # ---------------- phase 1: QKV projection + wT ----------------
            for sc in range(N_SCHUNKS):
                if sc == 0:
                    xbt_sb = None
                else:
                    xbt_sb = xin.tile([128, ST_PER_CHUNK, 8, 128], BF16,
                                      tag="xbt")
                    nc.sync.dma_start(out=xbt_sb[:], in_=xbt_v[sc])

                yqk_tiles = [ywork.tile([128, 1024], BF16, tag="yqk",
                                        name=f"yqk{sc}_{st}")
                             for st in range(ST_PER_CHUNK)]

                def evict(st, nh, ps_y, sc=sc):
                    if nh < 2:
                        nc.vector.tensor_tensor(
                            out=yqk_tiles[st][:, nh * 512:(nh + 1) * 512],
                            in0=ps_y[:],
                            in1=b_bc[:, nh * 512:(nh + 1) * 512],
                            op=mybir.AluOpType.add,
                        )
                    else:
                        nc.vector.tensor_tensor(
                            out=v_sb[:, sc * ST_PER_CHUNK + st, :],
                            in0=ps_y[:],
                            in1=b_bc[:, 1024:1536],
                            op=mybir.AluOpType.add,
                        )

                n_drain = [1, 1, 2, 1, 1, 2]
                if sc == 0:
                    # match the startup ring FIFO: Q,Q,K,K,V,V
                    group_order = [(0, 0), (1, 0), (0, 1), (1, 1),
                                   (0, 2), (1, 2)]
                else:
                    group_order = [(st, nh) for st in range(ST_PER_CHUNK)
                                   for nh in range(3)]
                for gi, (st, nh) in enumerate(group_order):
                    if True:
                        ps_y = psacc.tile([128, 512], F32, tag="acc")
                        for kb in range(8):
                            lhsT = (c0st_sb[st][:, kb, :] if sc == 0
                                    else xbt_sb[:, st, kb, :])
                            nc.tensor.matmul(
                                ps_y[:],
                                lhsT=lhsT,
                                rhs=w_ap(nh, kb),
                                start=(kb == 0),
                                stop=(kb == 7),
                            )
                        evict(st, nh, ps_y)
                        drain_wt(n_drain[gi])

                if sc == 2:
                    # Gate the w2t/b2 loads on chunk-2 progress via a WAR
                    # data dependency (dummy reads of the DMA dest regions
                    # that also read a chunk-2 tile) -- plain engine-order
                    # blockers get hoisted by the scheduler and the 2.25MB
                    # lands in the HBM-saturated startup window.
                    nc.vector.tensor_tensor(
                        out=blk_sb[:, 0:1], in0=w2_sb[0:1, 0, 0:1],
                        in1=yqk_tiles[0][0:1, 0:1], op=mybir.AluOpType.add)
                    nc.vector.tensor_tensor(
                        out=blk_sb[:, 0:1], in0=w2_sb[0:1, 4, 0:1],
                        in1=yqk_tiles[0][0:1, 0:1], op=mybir.AluOpType.add)
                    nc.vector.tensor_tensor(
                        out=blk_sb[:, 1:2], in0=b2_bc[0:1, 0:1],
                        in1=yqk_tiles[0][0:1, 0:1], op=mybir.AluOpType.add)
                    nc.scalar.dma_start(out=w2_sb[:, 0:4, :], in_=w2t_v[:, 0:4, :])
                    nc.gpsimd.dma_start(out=w2_sb[:, 4:8, :], in_=w2t_v[:, 4:8, :])
                    nc.gpsimd.dma_start(
                        out=b2_bc[:], in_=b2[:])

                # wT accumulation packed per head-pair: lhsT = K cols of the
                # pair [128s, 128], rhs = Q cols [128s, 128].  Off-diagonal
                # 64x64 blocks are junk cross-head products, never read.
                # Queued here, drained into the NEXT chunk's dense stream.
                for st in range(ST_PER_CHUNK):
                    for p in range(4):
                        # stop=True on each pair's final accumulation (the
                        # last s-tile) -- every column-block group needs its
                        # own stop.
                        push_wt(yqk_tiles[st], p,
                                is_last=(sc == N_SCHUNKS - 1
                                         and st == ST_PER_CHUNK - 1))

            # ---------------- phase 2: exp; per-pair G -> Z -> F ----------
            # Chunk 7's queued wT matmuls must be emitted BEFORE exp so the
            # exp read-after-write dependency covers them.
            drain_wt(len(wt_queue))

            # exp on ACT overlaps the first G matmul group.
            expw_sb = attn.tile([128, 4, 128], BF16)
            nc.vector.memset(expw_sb[:], 0.0)
            for hl in range(NH):
                p, g = hl // 2, hl % 2
                nc.scalar.activation(
                    out=expw_sb[g * 64:(g + 1) * 64, p, g * 64:(g + 1) * 64],
                    in_=psum_wt[g * 64:(g + 1) * 64,
                                p * 128 + g * 64:p * 128 + (g + 1) * 64],
                    func=mybir.ActivationFunctionType.Exp,
                    scale=SCALE,
                )

            # Z and rZ for ALL pairs up front (they only need psum_wt), so
            # the F(p) matmuls have no late dependencies and the scheduler
            # keeps them interleaved right after their G(p) groups instead
            # of batching every store into the kernel tail.
            ps_z = pszr.tile([128, 4], F32, tag="z")
            for p in range(4):
                nc.tensor.matmul(
                    ps_z[:, p:p + 1],
                    lhsT=expw_sb[:, p, :],
                    rhs=ones_sb[:],
                    start=(p == 0),
                    stop=(p == 3),
                    skip_group_check=True,
                )
            rz_sb = attn.tile([128, 4], F32)
            nc.vector.reciprocal(rz_sb[:], ps_z[:])

            g_sb = attn.tile([128, 16, 512], BF16)      # [(e2), p*4+half*2+nh, n]
            store_queues = [nc.sync, nc.scalar, nc.gpsimd]
            store_state = [0]

            def emit_F_half(p, half, split_store=False):
                # F(p,half) = expw(p)^T x G'(p,half); evictions scale by rZ,
                # split across ACT (nh=0) and DVE (nh=1) so the psum rotation
                # never waits a single serialized eviction engine.  The 4th
                # psum rides the pszr rotation.  split_store (the final
                # half) issues two [128,512] stores on separate queues so
                # the kernel-tail DMA is half-sized and starts earlier.
                f_sb = fout.tile([128, 1024], BF16, tag="f",
                                 name=f"f{p}_{half}")
                for nh in range(2):
                    if half == 1 and nh == 1:
                        ps_f = pszr.tile([128, 512], F32, tag="z",
                                         name=f"psf{p}_{half}_{nh}")
                    else:
                        ps_f = psacc.tile([128, 512], F32, tag="acc",
                                          name=f"psf{p}_{half}_{nh}")
                    nc.tensor.matmul(
                        ps_f[:],
                        lhsT=expw_sb[:, p, :],
                        rhs=g_sb[:, p * 4 + half * 2 + nh, :],
                    )
                    if nh == 0:
                        nc.scalar.activation(
                            out=f_sb[:, 0:512],
                            in_=ps_f[:],
                            func=mybir.ActivationFunctionType.Copy,
                            scale=rz_sb[:, p:p + 1],
                        )
                        if split_store:
                            eng = store_queues[store_state[0] % 3]
                            store_state[0] += 1
                            eng.dma_start(
                                out=out_v[p, :, :, half, 0:512],
                                in_=f_sb[:, 0:512],
                            )
                    elif p == 3 and half == 1:
                        nc.vector.scalar_tensor_tensor(
                            out=f_sb[:, 512:1024],
                            in0=ps_f[:],
                            scalar=rz_sb[:, p:p + 1],
                            in1=b2_bc[:, 512:1024],
                            op0=mybir.AluOpType.mult,
                            op1=mybir.AluOpType.add,
                        )
                    else:
                        # ACT, not DVE: the scheduler sinks DVE f-evicts
                        # behind the psg-critical g-evicts, which stalls the
                        # psacc rotation and bunches F matmuls + stores into
                        # the kernel tail.  ACT is idle through the G pass.
                        nc.scalar.activation(
                            out=f_sb[:, 512:1024],
                            in_=ps_f[:],
                            func=mybir.ActivationFunctionType.Copy,
                            scale=rz_sb[:, p:p + 1],
                        )
                if split_store:
                    eng = store_queues[store_state[0] % 3]
                    store_state[0] += 1
                    eng.dma_start(
                        out=out_v[p, :, :, half, 512:1024],
                        in_=f_sb[:, 512:1024],
                    )
                else:
                    eng = store_queues[store_state[0] % 3]
                    store_state[0] += 1
                    eng.dma_start(
                        out=out_v[p, :, :, half, :],
                        in_=f_sb[:],
                    )

            # G_{p,half}[e2, n] = sum_j V[half*1024+j, p*128+e2] W2T[j, n]
            # (softmax-independent dense PE work; b2 folded in on evict).
            # F(p,half) is emitted between the nh-groups of the NEXT G
            # half, one half behind, so its g_sb / rz deps are satisfied
            # and the 8 output stores spread evenly through the G pass;
            # only F(3,1) (split store) tails after the last G group.
            fq = []
            for p in range(4):
                for half in range(2):
                    for nh in range(2):
                        ps_g = psg.tile([128, 512], F32, tag="psg",
                                        name=f"psg{p}_{half}_{nh}")
                        for jb in range(8):
                            nc.tensor.matmul(
                                ps_g[:],
                                lhsT=v_sb[:, half * 8 + jb,
                                          p * 128:(p + 1) * 128],
                                rhs=w2_sb[:, jb, nh * 512:(nh + 1) * 512],
                                start=(jb == 0),
                                stop=(jb == 7),
                            )
                        if p == 3 and half == 1 and nh == 1:
                            # plain copy on ACT (parallel with DVE's nh0
                            # eviction); b2 is restored in this block's F
                            # eviction instead.
                            nc.scalar.activation(
                                out=g_sb[:, 15, :],
                                in_=ps_g[:],
                                func=mybir.ActivationFunctionType.Copy,
                            )
                        else:
                            nc.vector.tensor_tensor(
                                out=g_sb[:, p * 4 + half * 2 + nh, :],
                                in0=ps_g[:],
                                in1=b2_bc[:, nh * 512:(nh + 1) * 512],
                                op=mybir.AluOpType.add,
                            )
                        if nh == 0 and fq:
                            # priority 0: offset-based priority still let the
                            # scheduler sink half the F matmuls + stores into
                            # the kernel tail.
                            with tc.high_priority():
                                emit_F_half(*fq.pop(0))
                    fq.append((p, half))
            emit_F_half(3, 1, split_store=True)

    nc.finalize()
    return nc


_NC_CACHE = None


def _get_nc():
    global _NC_CACHE
    if _NC_CACHE is None:
        _NC_CACHE = build_nc()
    return _NC_CACHE


def _shard_inputs(X, W1, b1, W2, b2):
    X = np.asarray(X, np.float32)
    W1 = np.asarray(W1, np.float32)
    b1 = np.asarray(b1, np.float32)
    W2 = np.asarray(W2, np.float32)
    b2 = np.asarray(b2, np.float32)

    w2t = np.ascontiguousarray(W2.T).astype(ml_dtypes.bfloat16)
    b2r = np.ascontiguousarray(
        np.broadcast_to(b2.reshape(1, 1024), (128, 1024))).astype(ml_dtypes.bfloat16)
    # chunk-major xbt with per-chunk col order [st, kb, 128]:
    # row c*128+p, col st*1024+kb*128+s'' = X[b].T[kb*128+p, c*256+st*128+s'']
    xbts = []
    for b in range(B):
        xt = X[b].T.astype(ml_dtypes.bfloat16)            # [1024, 2048]
        xc = np.ascontiguousarray(
            xt.reshape(8, 128, 8, 2, 128).transpose(2, 1, 3, 0, 4)
            .reshape(1024, 2048))
        xbts.append(xc)

    def pack_w(block):
        # [1024, 512] -> [128, 4096]: row p, col kb*512+c = block[kb*128+p, c]
        # (8KB-contiguous partition rows for fat DMA descriptors)
        return np.ascontiguousarray(
            block.reshape(8, 128, 512).transpose(1, 0, 2).reshape(128, 4096)
        ).astype(ml_dtypes.bfloat16)

    per_hg = []
    for hg in range(2):
        heads = range(NH * hg, NH * hg + NH)
        rows = np.concatenate(
            [np.arange(h * DH, (h + 1) * DH) for h in heads]
            + [D + np.arange(h * DH, (h + 1) * DH) for h in heads]
            + [2 * D + np.arange(h * DH, (h + 1) * DH) for h in heads])
        wqkvt = np.ascontiguousarray(W1[rows].T)               # [1024, 1536]
        wq = pack_w(wqkvt[:, 0:512])
        wk = pack_w(wqkvt[:, 512:1024])
        wv = pack_w(wqkvt[:, 1024:1536])
        bqkv = np.ascontiguousarray(b1[rows].reshape(1, 1536)).astype(ml_dtypes.bfloat16)
        per_hg.append((wq, wk, wv, bqkv))

    in_maps = []
    for c in range(8):
        b, hg = c // 2, c % 2
        wq, wk, wv, bqkv = per_hg[hg]
        in_maps.append({
            "xbt": xbts[b], "wq": wq, "wk": wk, "wv": wv, "bqkv": bqkv,
            "w2t": w2t, "b2": b2r,
        })
    return in_maps


def run(X, W1, b1, W2, b2, **run_kwargs):
    """Returns (full_output, BassKernelResults)."""
    nc = _get_nc()
    in_maps = _shard_inputs(X, W1, b1, W2, b2)
    res = run_bass_kernel_spmd(nc, in_maps, core_ids=list(range(8)), **run_kwargs)
    full = np.empty((B, S, D), np.float32)
    for c in range(8):
        b, hg = c // 2, c % 2
        full[b, hg * 1024:(hg + 1) * 1024, :] = np.asarray(
            res.results[c]["out"]).astype(np.float32)
    return full, res


def kernel(X, W1, b1, W2, b2):
    return run(X, W1, b1, W2, b2)[0]

